# revision 45
# baseline (speedup 1.0000x reference)
"""MoE feed-forward (top-2 routing, 8 experts) on 8 Trainium2 NeuronCores.

Expert-parallel sharding: host computes the (tiny) router + argsort
permutation exactly as the reference does, gathers each expert's token
chunk, and sends chunk e + expert e's weights to core e. Each core runs
a dense FFN: y = gelu(x @ w1) @ w2, scaled by the per-row gate weight.
Host then inverts the permutation and sums the top-2 contributions.

Device kernel (per core, SPMD), v3:
  - mm1 uses one level of Strassen (7 products instead of 8 on 2x2x2
    blocking of [4096f x 1024d] @ [1024d x 2048c]): 12.5% fewer PE
    cycles. The 7 stationary operand combinations (w1-side) and 7 moving
    combinations (x-side) are precomputed on the host and shipped as
    inputs; the P1..P7 -> H block recombination runs on the (otherwise
    idle) DVE, reading the 7 PSUM banks and writing fp32 SBUF scratch
    that ScalarE gelus into the bf16 h tile.
  - warmup matmuls on a memset tile during the initial DMA window so the
    PE HAM clock-gate reaches 8/8 before real work arrives.
  - mm2 dense in bf16 (w2 resident in SBUF): y[c,d] += h^T @ w2,
    accumulated over all 32 f-tiles in PSUM; gate scale applied during
    the PSUM->SBUF drain on ScalarE; y written out in bf16 (host
    combines in fp32).
  - phase order: mm1 units cs=0 (produces h for c-blocks 0 and 2) ->
    mm2 cb0+cb2 -> mm1 units cs=1 -> mm2 cb1+cb3.
"""

import os
import sys

# Register the CPU jax backend alongside whatever platform is configured,
# so the router can be computed on CPU (bit-exact with a CPU-evaluated
# reference). A no-op if jax is already initialized or cpu is present.
if "jax" not in sys.modules:
    _jp = os.environ.get("JAX_PLATFORMS", "")
    if _jp and "cpu" not in _jp.split(","):
        os.environ["JAX_PLATFORMS"] = _jp + ",cpu"

import numpy as np

# Static problem config
B, T, D, FF, E, TOP_K = 4, 2048, 1024, 4096, 8, 2
N = B * T                    # 8192 tokens
NE = N * TOP_K               # 16384 expanded rows
C_PER = NE // E              # 2048 rows per core / expert chunk
P = 128
FT = FF // P                 # 32 f-tiles
DT = D // P                  # 8 d-tiles
NCB = 4                      # c-blocks per core
CB = C_PER // NCB            # 512 tokens per c-block
NCT = C_PER // P             # 16 c-tiles of 128
FTB = FT // 2                # 16 f-tiles per Strassen block row
SDT = 4                      # d-tiles per Strassen half (512/128)
NWARM = 48                   # warmup matmuls (FD=128) before data lands

_cache = {}


def _build_program(act_name="Gelu"):
    import concourse.mybir as mybir
    import concourse.tile as tile
    from concourse import bacc

    f32 = mybir.dt.float32
    bf16 = mybir.dt.bfloat16
    Act = mybir.ActivationFunctionType
    Alu = mybir.AluOpType

    nc = bacc.Bacc("TRN2", num_devices=E)
    # 7 stationary combos [ftb, dp, k, dt, fp] and 7 moving combos
    # [k, cs, dp, dt, c] — ordered so each DMA reads multi-KB
    # contiguous runs per partition (small descriptors tank HBM DMA)
    s_d = nc.dram_tensor("s7", [FTB, P, 7, SDT, P], bf16,
                         kind="ExternalInput")
    m_d = nc.dram_tensor("m7", [7, 2, P, SDT, CB], bf16,
                         kind="ExternalInput")
    w2_d = nc.dram_tensor("w2b", [FF, D], bf16, kind="ExternalInput")
    sw_d = nc.dram_tensor("swt", [P, NCT], f32, kind="ExternalInput")
    y_d = nc.dram_tensor("y", [C_PER, D], bf16, kind="ExternalOutput")

    with tile.TileContext(nc) as tc:
        with tc.tile_pool(name="const", bufs=1) as cpool, \
             tc.tile_pool(name="mp", bufs=1) as mp, \
             tc.tile_pool(name="sp", bufs=3) as sp, \
             tc.tile_pool(name="w2p", bufs=1) as w2p, \
             tc.tile_pool(name="hp", bufs=1) as hp, \
             tc.tile_pool(name="scr", bufs=1) as scr, \
             tc.tile_pool(name="yop", bufs=2) as yop, \
             tc.tile_pool(name="ps", bufs=1, space="PSUM") as ps:

            # ---- warmup: keep the PE busy while the first DMAs land ----
            wmt = cpool.tile([P, P], bf16, tag="warm_sb")
            nc.gpsimd.memset(wmt, 0.0)
            wps = ps.tile([P, CB], f32, tag="ps0", name="warm_ps")
            for _ in range(NWARM):
                nc.tensor.matmul(wps[:, :P], wmt, wmt, start=True, stop=True)

            swt = cpool.tile([P, NCT], f32, tag="swt")
            nc.scalar.dma_start(out=swt, in_=sw_d.ap())

            # moving combos for one cs: 28 resident [P, CB] tiles,
            # reloaded between phases (bufs=1: refill waits for the
            # last phase-A reader, streams during the mm2 phase)
            m_t = {}

            def load_m(cs):
                for k in range(7):
                    t = mp.tile([P, SDT, CB], bf16, tag=f"m{k}",
                                name=f"m{k}_{cs}")
                    nc.scalar.dma_start(out=t, in_=m_d.ap()[k, cs])
                    m_t[k] = t

            # w2 resident as 4 chunk tiles [P, 8, D], loaded in 0.5MB
            # slices on the sync queue interleaved with the stationary
            # stream: queue FIFO order actually delays them past the
            # startup burst (a queue with no other work issues its DMAs
            # immediately, whatever the program position)
            w2src = w2_d.ap().rearrange(
                "(q f8 p) d -> q p f8 d", q=4, f8=FT // 4, p=P)
            w2t = [w2p.tile([P, FT // 4, D], bf16, tag=f"w2_{i}",
                            name=f"w2_{i}") for i in range(4)]

            def load_w2_part(j):
                q, part = j // 4, j % 4
                nc.sync.dma_start(
                    out=w2t[q][:, 2 * part:2 * part + 2, :],
                    in_=w2src[q][:, 2 * part:2 * part + 2, :])

            def stt(out, in0, scalar, in1):
                nc.vector.scalar_tensor_tensor(
                    out=out, in0=in0, scalar=scalar, in1=in1,
                    op0=Alu.mult, op1=Alu.add)

            load_m(0)
            for cs in range(2):
                h_lo = hp.tile([P, FT, CB], bf16, tag="h_lo",
                               name=f"h_lo_{cs}")
                h_hi = hp.tile([P, FT, CB], bf16, tag="h_hi",
                               name=f"h_hi_{cs}")
                for ftb in range(FTB):
                    # all 7 stationary combos for this ftb in one DMA
                    s_t = sp.tile([P, 7, SDT, P], bf16, tag="s",
                                  name=f"s_{cs}_{ftb}")
                    nc.sync.dma_start(out=s_t, in_=s_d.ap()[ftb])
                    # second half of phase A: HBM is free of the m-load
                    # burst by then, and mm2 needs w2 only at ~110us
                    if cs == 0 and 6 <= ftb <= 13:
                        load_w2_part(2 * (ftb - 6))
                        load_w2_part(2 * (ftb - 6) + 1)
                    p_t = []
                    for k in range(7):
                        pk = ps.tile([P, CB], f32, tag=f"ps{k}",
                                     name=f"p{k}_{ftb}")
                        for dt in range(SDT):
                            nc.tensor.matmul(pk, s_t[:, k, dt, :],
                                             m_t[k][:, dt, :],
                                             start=(dt == 0),
                                             stop=(dt == SDT - 1))
                        p_t.append(pk)
                    # ISA: a 2-input DVE op may read at most ONE PSUM
                    # operand, so P0/P1/P2 are copied to SBUF and every
                    # combine reads (sbuf, psum).
                    def st(tag):
                        return scr.tile([P, CB], f32, tag=tag, name=tag)

                    def add_ps(eng, out, sb, pk, sign=1.0):
                        # out = sb + sign*pk   (pk in PSUM)
                        op1 = Alu.add if sign > 0 else Alu.subtract
                        eng.scalar_tensor_tensor(
                            out=out, in0=sb, scalar=1.0, in1=pk,
                            op0=Alu.mult, op1=op1)

                    # combines split across DVE and ScalarE (GpSimd has
                    # no PSUM port): 11 DVE passes (~6.6us) would exceed
                    # the unit's PE time and stall PSUM-bank reuse
                    dstA = h_lo[:, ftb, :]
                    dstC = h_lo[:, FTB + ftb, :]
                    dstB = h_hi[:, ftb, :]
                    dstD = h_hi[:, FTB + ftb, :]

                    # H11 = P0+P3-P4+P6; H21 = P1+P3; H12 = P2+P4;
                    # H22 = P0-P1+P2+P5.
                    # DVE is strict FIFO: passes are ordered by when
                    # their newest PSUM input completes (P_k done at
                    # ~0.86*(k+1)us into the unit), so no pass
                    # head-of-line-blocks ready work behind it. hsA
                    # (needs P6, the last product) therefore goes last.
                    u0, u1, u2 = st("u0"), st("u1"), st("u2")
                    nc.vector.tensor_copy(u0, p_t[0])
                    nc.vector.tensor_copy(u1, p_t[1])
                    nc.vector.tensor_copy(u2, p_t[2])
                    v1, w1_, hsA = st("v1"), st("w1"), st("hsA")
                    hsC, hsB = st("hsC"), st("hsB")
                    v2, v3, hsD = st("v2"), st("v3"), st("hsD")
                    add_ps(nc.vector, v1, u0, p_t[3])
                    add_ps(nc.vector, hsC, u1, p_t[3])
                    nc.scalar.activation(dstC, hsC,
                                         getattr(Act, act_name))
                    add_ps(nc.vector, w1_, v1, p_t[4], -1.0)
                    add_ps(nc.vector, hsB, u2, p_t[4])
                    nc.scalar.activation(dstB, hsB,
                                         getattr(Act, act_name))
                    add_ps(nc.vector, v2, u0, p_t[1], -1.0)
                    add_ps(nc.vector, v3, v2, p_t[2])
                    add_ps(nc.vector, hsD, v3, p_t[5])
                    nc.scalar.activation(dstD, hsD,
                                         getattr(Act, act_name))
                    add_ps(nc.vector, hsA, w1_, p_t[6])
                    nc.scalar.activation(dstA, hsA,
                                         getattr(Act, act_name))

                if cs == 0:
                    # refill the moving combos for phase B; the WAR
                    # dependency on phase A's last reader makes these
                    # stream during the first mm2 phase
                    load_m(1)

                # mm2 for c-blocks cs (from h_lo) and 2+cs (from h_hi)
                for cb, h_t in ((cs, h_lo), (2 + cs, h_hi)):
                    for ct in range(CB // P):
                        g = cb * (CB // P) + ct
                        for db in range(2):
                            d0 = db * (D // 2)
                            yps = ps.tile([P, D // 2], f32,
                                          tag=f"ps{(ct * 2 + db) % 4}",
                                          name="yps")
                            for ft in range(FT):
                                nc.tensor.matmul(
                                    yps, h_t[:, ft, ct * P:(ct + 1) * P],
                                    w2t[ft // (FT // 4)][:, ft % (FT // 4),
                                                         d0:d0 + D // 2],
                                    start=(ft == 0), stop=(ft == FT - 1))
                            yo = yop.tile([P, D // 2], bf16, tag="yo",
                                          name="yo")
                            nc.scalar.activation(yo, yps, Act.Copy,
                                                 scale=swt[:, g:g + 1])
                            # per-half store: the last store overlaps the
                            # final matmul group instead of trailing it
                            # (scalar queue keeps sync free for the next
                            # phase's stationary-combo prefetch)
                            nc.scalar.dma_start(
                                out=y_d.ap()[g * P:(g + 1) * P,
                                             d0:d0 + D // 2], in_=yo)
    nc.compile()
    return nc


def _get_program():
    if "nc" not in _cache:
        _cache["nc"] = _build_program()
    return _cache["nc"]


def _routing(xf, router_w):
    """Replicate the reference gating bit-exactly where it matters.

    Returns (rev, sw, sort_idx). The top-k *selection* must match the
    reference exactly (it is discrete); we therefore compute the router
    logits with jax when available, mirroring reference.py. The softmax
    and sort bookkeeping is continuous or exactly replicable in numpy.
    """
    try:
        import jax
        import jax.numpy as jnp

        def _gate():
            logits = jnp.asarray(xf) @ jnp.asarray(router_w).T
            return jax.lax.top_k(logits, TOP_K)

        try:
            cpu = jax.devices("cpu")[0]
            with jax.default_device(cpu):
                tv, ti = _gate()
        except Exception:
            tv, ti = _gate()
        topv = np.asarray(tv, dtype=np.float32)
        topi = np.asarray(ti)
    except Exception:
        logits = xf @ router_w.T
        # top-2 with jax tie-breaking (lower index wins)
        i0 = np.argmax(logits, axis=-1)
        v0 = np.take_along_axis(logits, i0[:, None], axis=-1)[:, 0]
        masked = logits.copy()
        np.put_along_axis(masked, i0[:, None], -np.inf, axis=-1)
        i1 = np.argmax(masked, axis=-1)
        v1 = np.take_along_axis(masked, i1[:, None], axis=-1)[:, 0]
        topi = np.stack([i0, i1], axis=-1)
        topv = np.stack([v0, v1], axis=-1).astype(np.float32)

    # softmax over the two gate logits, float32
    m = topv.max(axis=-1, keepdims=True)
    e = np.exp(topv - m, dtype=np.float32)
    topw = (e / e.sum(axis=-1, keepdims=True)).astype(np.float32)

    idx_flat = topi.reshape(-1)
    w_flat = topw.reshape(-1)
    # stable argsort of integer keys is uniquely determined by the keys
    sort_idx = np.argsort(idx_flat, kind="stable")
    src = np.repeat(np.arange(N), TOP_K)
    rev = src[sort_idx]
    sw = w_flat[sort_idx]
    return rev, sw, sort_idx


def _ensure_axon_hooks():
    """Make `antenv.axon_hooks` importable so run_bass_kernel_spmd's
    trace path degrades gracefully (or works, if the axon boot shim is
    available) instead of crashing on ImportError."""
    try:
        import antenv.axon_hooks  # noqa: F401
        return
    except ImportError:
        pass
    import sys
    import types
    mod = types.ModuleType("antenv.axon_hooks")
    state = {"hook": None}
    mod.set_axon_ntff_profile_hook = lambda h: state.update(hook=h)
    mod.get_axon_ntff_profile_hook = lambda: state["hook"]
    try:
        import antenv
        sys.modules["antenv.axon_hooks"] = mod
        antenv.axon_hooks = mod
    except ImportError:
        return
    try:
        from trn_agent_boot.trn_boot import _ntff_profile_via_ctypes
        h = _ntff_profile_via_ctypes("/opt/axon/libaxon_pjrt.so")
        if h is not None:
            mod.set_axon_ntff_profile_hook(h)
            import concourse.bass_utils as bu
            bu.upload_artifacts = lambda tmpdir: "local://" + str(tmpdir)
    except Exception:
        pass


def _strassen_operands(W1, X, bf16):
    """Host-side level-1 Strassen operand combos for H = W1^T X.

    W1: [D, FF] fp32; X: [D, C_PER] fp32. Returns (s_host, m_host):
      s_host [7, FTB, P, SDT, P] bf16 (stationary, w1 side)
      m_host [7, SDT, 2, P, CB] bf16 (moving, x side)
    """
    A = W1.T                                   # [FF, D]
    hf, hd, hc = FF // 2, D // 2, C_PER // 2
    A11, A12 = A[:hf, :hd], A[:hf, hd:]
    A21, A22 = A[hf:, :hd], A[hf:, hd:]
    B11, B12 = X[:hd, :hc], X[:hd, hc:]
    B21, B22 = X[hd:, :hc], X[hd:, hc:]
    S = [A11 + A22, A21 + A22, A11, A22, A11 + A12, A21 - A11, A12 - A22]
    M = [B11 + B22, B11, B12 - B22, B21 - B11, B22, B11 + B12, B21 + B22]
    # [k, ftb, dp, dt, fp] -> [ftb, dp, k, dt, fp] (contiguous per part.)
    s_host = np.stack([
        Sk.astype(bf16).reshape(FTB, P, SDT, P).transpose(0, 3, 2, 1)
        for Sk in S]).transpose(1, 2, 0, 3, 4)
    # [k, dt, cs, dp, c] -> [k, cs, dp, dt, c]
    m_host = np.stack([
        Mk.astype(bf16).reshape(SDT, P, 2, CB).transpose(0, 2, 1, 3)
        for Mk in M]).transpose(0, 2, 3, 1, 4)
    return np.ascontiguousarray(s_host), np.ascontiguousarray(m_host)


def kernel(x, router_w, w1, w2):
    import ml_dtypes
    from concourse import bass_utils
    _ensure_axon_hooks()

    xf = np.ascontiguousarray(x.reshape(-1, D), dtype=np.float32)
    rev, sw, sort_idx = _routing(xf, router_w)

    nc = _get_program()

    in_maps = []
    for e in range(E):
        rows = rev[e * C_PER:(e + 1) * C_PER]
        xct = np.ascontiguousarray(xf[rows].T)            # [D, C_PER] fp32
        s_host, m_host = _strassen_operands(w1[e], xct, ml_dtypes.bfloat16)
        w2b = np.ascontiguousarray(w2[e].astype(ml_dtypes.bfloat16))
        swt = np.ascontiguousarray(
            sw[e * C_PER:(e + 1) * C_PER].reshape(NCT, P).T)
        in_maps.append({"s7": s_host, "m7": m_host, "w2b": w2b,
                        "swt": swt})

    r = bass_utils.run_bass_kernel_spmd(nc, in_maps, core_ids=list(range(E)))
    _cache["last_result"] = r

    y_sorted = np.empty((NE, D), dtype=np.float32)
    for e in range(E):
        y_sorted[e * C_PER:(e + 1) * C_PER] = r.results[e]["y"].astype(
            np.float32)

    # invert the permutation and combine the top-2 contributions
    y_expanded = np.empty_like(y_sorted)
    y_expanded[sort_idx] = y_sorted
    out = y_expanded.reshape(N, TOP_K, D).sum(axis=1)
    return out.reshape(B, T, D)


# revision 49
# speedup vs baseline: 1.0078x; 1.0078x over previous
"""MoE feed-forward (top-2 routing, 8 experts) on 8 Trainium2 NeuronCores.

Expert-parallel sharding: host computes the (tiny) router + argsort
permutation exactly as the reference does, gathers each expert's token
chunk, and sends chunk e + expert e's weights to core e. Each core runs
a dense FFN: y = gelu(x @ w1) @ w2, scaled by the per-row gate weight.
Host then inverts the permutation and sums the top-2 contributions.

Device kernel (per core, SPMD), v3:
  - mm1 uses one level of Strassen (7 products instead of 8 on 2x2x2
    blocking of [4096f x 1024d] @ [1024d x 2048c]): 12.5% fewer PE
    cycles. The 7 stationary operand combinations (w1-side) and 7 moving
    combinations (x-side) are precomputed on the host and shipped as
    inputs; the P1..P7 -> H block recombination runs on the (otherwise
    idle) DVE, reading the 7 PSUM banks and writing fp32 SBUF scratch
    that ScalarE gelus into the bf16 h tile.
  - warmup matmuls on a memset tile during the initial DMA window so the
    PE HAM clock-gate reaches 8/8 before real work arrives.
  - mm2 dense in bf16 (w2 resident in SBUF): y[c,d] += h^T @ w2,
    accumulated over all 32 f-tiles in PSUM; gate scale applied during
    the PSUM->SBUF drain on ScalarE; y written out in bf16 (host
    combines in fp32).
  - phase order: mm1 units cs=0 (produces h for c-blocks 0 and 2) ->
    mm2 cb0+cb2 -> mm1 units cs=1 -> mm2 cb1+cb3.
"""

import os
import sys

# Register the CPU jax backend alongside whatever platform is configured,
# so the router can be computed on CPU (bit-exact with a CPU-evaluated
# reference). A no-op if jax is already initialized or cpu is present.
if "jax" not in sys.modules:
    _jp = os.environ.get("JAX_PLATFORMS", "")
    if _jp and "cpu" not in _jp.split(","):
        os.environ["JAX_PLATFORMS"] = _jp + ",cpu"

import numpy as np

# Static problem config
B, T, D, FF, E, TOP_K = 4, 2048, 1024, 4096, 8, 2
N = B * T                    # 8192 tokens
NE = N * TOP_K               # 16384 expanded rows
C_PER = NE // E              # 2048 rows per core / expert chunk
P = 128
FT = FF // P                 # 32 f-tiles
DT = D // P                  # 8 d-tiles
NCB = 4                      # c-blocks per core
CB = C_PER // NCB            # 512 tokens per c-block
NCT = C_PER // P             # 16 c-tiles of 128
FTB = FT // 2                # 16 f-tiles per Strassen block row
SDT = 4                      # d-tiles per Strassen half (512/128)
NWARM = 48                   # warmup matmuls (FD=128) before data lands

_cache = {}


def _build_program(act_name="Gelu"):
    import concourse.mybir as mybir
    import concourse.tile as tile
    from concourse import bacc

    f32 = mybir.dt.float32
    bf16 = mybir.dt.bfloat16
    Act = mybir.ActivationFunctionType
    Alu = mybir.AluOpType

    nc = bacc.Bacc("TRN2", num_devices=E)
    # 7 stationary combos [ftb, dp, k, dt, fp] and 7 moving combos
    # [k, cs, dp, dt, c] — ordered so each DMA reads multi-KB
    # contiguous runs per partition (small descriptors tank HBM DMA)
    s_d = nc.dram_tensor("s7", [FTB, P, 7, SDT, P], bf16,
                         kind="ExternalInput")
    m_d = nc.dram_tensor("m7", [7, 2, P, SDT, CB], bf16,
                         kind="ExternalInput")
    w2_d = nc.dram_tensor("w2b", [FF, D], bf16, kind="ExternalInput")
    sw_d = nc.dram_tensor("swt", [P, NCT], f32, kind="ExternalInput")
    y_d = nc.dram_tensor("y", [C_PER, D], bf16, kind="ExternalOutput")

    with tile.TileContext(nc) as tc:
        with tc.tile_pool(name="const", bufs=1) as cpool, \
             tc.tile_pool(name="mp", bufs=1) as mp, \
             tc.tile_pool(name="sp", bufs=3) as sp, \
             tc.tile_pool(name="w2p", bufs=1) as w2p, \
             tc.tile_pool(name="hp", bufs=1) as hp, \
             tc.tile_pool(name="scr", bufs=1) as scr, \
             tc.tile_pool(name="yop", bufs=2) as yop, \
             tc.tile_pool(name="ps", bufs=1, space="PSUM") as ps:

            # ---- warmup: keep the PE busy while the first DMAs land ----
            wmt = cpool.tile([P, P], bf16, tag="warm_sb")
            nc.gpsimd.memset(wmt, 0.0)
            wps = ps.tile([P, CB], f32, tag="ps0", name="warm_ps")
            for _ in range(NWARM):
                nc.tensor.matmul(wps[:, :P], wmt, wmt, start=True, stop=True)

            swt = cpool.tile([P, NCT], f32, tag="swt")
            nc.scalar.dma_start(out=swt, in_=sw_d.ap())

            # moving combos for one cs: 28 resident [P, CB] tiles,
            # reloaded between phases (bufs=1: refill waits for the
            # last phase-A reader, streams during the mm2 phase)
            m_t = {}

            def load_m(cs):
                for k in range(7):
                    t = mp.tile([P, SDT, CB], bf16, tag=f"m{k}",
                                name=f"m{k}_{cs}")
                    nc.scalar.dma_start(out=t, in_=m_d.ap()[k, cs])
                    m_t[k] = t

            # w2 resident as 4 chunk tiles [P, 8, D], loaded in 0.5MB
            # slices on the sync queue interleaved with the stationary
            # stream: queue FIFO order actually delays them past the
            # startup burst (a queue with no other work issues its DMAs
            # immediately, whatever the program position)
            w2src = w2_d.ap().rearrange(
                "(q f8 p) d -> q p f8 d", q=4, f8=FT // 4, p=P)
            w2t = [w2p.tile([P, FT // 4, D], bf16, tag=f"w2_{i}",
                            name=f"w2_{i}") for i in range(4)]

            def load_w2_part(j):
                q, part = j // 4, j % 4
                nc.sync.dma_start(
                    out=w2t[q][:, 2 * part:2 * part + 2, :],
                    in_=w2src[q][:, 2 * part:2 * part + 2, :])

            def stt(out, in0, scalar, in1):
                nc.vector.scalar_tensor_tensor(
                    out=out, in0=in0, scalar=scalar, in1=in1,
                    op0=Alu.mult, op1=Alu.add)

            load_m(0)
            for cs in range(2):
                h_lo = hp.tile([P, FT, CB], bf16, tag="h_lo",
                               name=f"h_lo_{cs}")
                h_hi = hp.tile([P, FT, CB], bf16, tag="h_hi",
                               name=f"h_hi_{cs}")
                for ftb in range(FTB):
                    # all 7 stationary combos for this ftb in one DMA
                    s_t = sp.tile([P, 7, SDT, P], bf16, tag="s",
                                  name=f"s_{cs}_{ftb}")
                    nc.sync.dma_start(out=s_t, in_=s_d.ap()[ftb])
                    # second half of phase A: HBM is free of the m-load
                    # burst by then, and mm2 needs w2 only at ~110us
                    if cs == 0 and 6 <= ftb <= 13:
                        load_w2_part(2 * (ftb - 6))
                        load_w2_part(2 * (ftb - 6) + 1)
                    p_t = []
                    for k in range(7):
                        pk = ps.tile([P, CB], f32, tag=f"ps{k}",
                                     name=f"p{k}_{ftb}")
                        for dt in range(SDT):
                            nc.tensor.matmul(pk, s_t[:, k, dt, :],
                                             m_t[k][:, dt, :],
                                             start=(dt == 0),
                                             stop=(dt == SDT - 1))
                        p_t.append(pk)
                    def st(tag):
                        return scr.tile([P, CB], f32, tag=tag, name=tag)

                    dstA = h_lo[:, ftb, :]
                    dstC = h_lo[:, FTB + ftb, :]
                    dstB = h_hi[:, ftb, :]
                    dstD = h_hi[:, FTB + ftb, :]

                    def add_ps(out, sb, pk, sign=1.0):
                        # out = sb + sign*pk   (pk in PSUM; a 2-input
                        # DVE op may read at most ONE PSUM operand)
                        op1 = Alu.add if sign > 0 else Alu.subtract
                        nc.vector.scalar_tensor_tensor(
                            out=out, in0=sb, scalar=1.0, in1=pk,
                            op0=Alu.mult, op1=op1)

                    # H11 = P0+P3-P4+P6; H21 = P1+P3; H12 = P2+P4;
                    # H22 = P0-P1+P2+P5.
                    # DVE is strict FIFO: passes are ordered by when
                    # their newest PSUM input completes (P_k done at
                    # ~0.86*(k+1)us into the unit), so no pass
                    # head-of-line-blocks ready work behind it. hsA
                    # (needs P6, the last product) therefore goes last.
                    u0, u1, u2 = st("u0"), st("u1"), st("u2")
                    nc.vector.tensor_copy(u0, p_t[0])
                    nc.vector.tensor_copy(u1, p_t[1])
                    nc.vector.tensor_copy(u2, p_t[2])
                    v1, w1_, hsA = st("v1"), st("w1"), st("hsA")
                    hsC, hsB = st("hsC"), st("hsB")
                    v2, v3, hsD = st("v2"), st("v3"), st("hsD")
                    add_ps(v1, u0, p_t[3])
                    add_ps(hsC, u1, p_t[3])
                    nc.scalar.activation(dstC, hsC,
                                         getattr(Act, act_name))
                    add_ps(w1_, v1, p_t[4], -1.0)
                    add_ps(hsB, u2, p_t[4])
                    nc.scalar.activation(dstB, hsB,
                                         getattr(Act, act_name))
                    add_ps(v2, u0, p_t[1], -1.0)
                    add_ps(v3, v2, p_t[2])
                    add_ps(hsD, v3, p_t[5])
                    nc.scalar.activation(dstD, hsD,
                                         getattr(Act, act_name))
                    add_ps(hsA, w1_, p_t[6])
                    nc.scalar.activation(dstA, hsA,
                                         getattr(Act, act_name))

                if cs == 0:
                    # refill the moving combos for phase B; the WAR
                    # dependency on phase A's last reader makes these
                    # stream during the first mm2 phase
                    load_m(1)

                # mm2 for c-blocks cs (from h_lo) and 2+cs (from h_hi)
                for cb, h_t in ((cs, h_lo), (2 + cs, h_hi)):
                    for ct in range(CB // P):
                        g = cb * (CB // P) + ct
                        for db in range(2):
                            d0 = db * (D // 2)
                            yps = ps.tile([P, D // 2], f32,
                                          tag=f"ps{(ct * 2 + db) % 4}",
                                          name="yps")
                            for ft in range(FT):
                                nc.tensor.matmul(
                                    yps, h_t[:, ft, ct * P:(ct + 1) * P],
                                    w2t[ft // (FT // 4)][:, ft % (FT // 4),
                                                         d0:d0 + D // 2],
                                    start=(ft == 0), stop=(ft == FT - 1))
                            yo = yop.tile([P, D // 2], bf16, tag="yo",
                                          name="yo")
                            nc.scalar.activation(yo, yps, Act.Copy,
                                                 scale=swt[:, g:g + 1])
                            # per-half store: the last store overlaps the
                            # final matmul group instead of trailing it
                            # (scalar queue keeps sync free for the next
                            # phase's stationary-combo prefetch)
                            nc.scalar.dma_start(
                                out=y_d.ap()[g * P:(g + 1) * P,
                                             d0:d0 + D // 2], in_=yo)
    nc.compile()
    return nc


def _get_program():
    if "nc" not in _cache:
        _cache["nc"] = _build_program()
    return _cache["nc"]


def _routing(xf, router_w):
    """Replicate the reference gating bit-exactly where it matters.

    Returns (rev, sw, sort_idx). The top-k *selection* must match the
    reference exactly (it is discrete); we therefore compute the router
    logits with jax when available, mirroring reference.py. The softmax
    and sort bookkeeping is continuous or exactly replicable in numpy.
    """
    try:
        import jax
        import jax.numpy as jnp

        def _gate():
            logits = jnp.asarray(xf) @ jnp.asarray(router_w).T
            return jax.lax.top_k(logits, TOP_K)

        try:
            cpu = jax.devices("cpu")[0]
            with jax.default_device(cpu):
                tv, ti = _gate()
        except Exception:
            tv, ti = _gate()
        topv = np.asarray(tv, dtype=np.float32)
        topi = np.asarray(ti)
    except Exception:
        logits = xf @ router_w.T
        # top-2 with jax tie-breaking (lower index wins)
        i0 = np.argmax(logits, axis=-1)
        v0 = np.take_along_axis(logits, i0[:, None], axis=-1)[:, 0]
        masked = logits.copy()
        np.put_along_axis(masked, i0[:, None], -np.inf, axis=-1)
        i1 = np.argmax(masked, axis=-1)
        v1 = np.take_along_axis(masked, i1[:, None], axis=-1)[:, 0]
        topi = np.stack([i0, i1], axis=-1)
        topv = np.stack([v0, v1], axis=-1).astype(np.float32)

    # softmax over the two gate logits, float32
    m = topv.max(axis=-1, keepdims=True)
    e = np.exp(topv - m, dtype=np.float32)
    topw = (e / e.sum(axis=-1, keepdims=True)).astype(np.float32)

    idx_flat = topi.reshape(-1)
    w_flat = topw.reshape(-1)
    # stable argsort of integer keys is uniquely determined by the keys
    sort_idx = np.argsort(idx_flat, kind="stable")
    src = np.repeat(np.arange(N), TOP_K)
    rev = src[sort_idx]
    sw = w_flat[sort_idx]
    return rev, sw, sort_idx


def _ensure_axon_hooks():
    """Make `antenv.axon_hooks` importable so run_bass_kernel_spmd's
    trace path degrades gracefully (or works, if the axon boot shim is
    available) instead of crashing on ImportError."""
    try:
        import antenv.axon_hooks  # noqa: F401
        return
    except ImportError:
        pass
    import sys
    import types
    mod = types.ModuleType("antenv.axon_hooks")
    state = {"hook": None}
    mod.set_axon_ntff_profile_hook = lambda h: state.update(hook=h)
    mod.get_axon_ntff_profile_hook = lambda: state["hook"]
    try:
        import antenv
        sys.modules["antenv.axon_hooks"] = mod
        antenv.axon_hooks = mod
    except ImportError:
        return
    try:
        from trn_agent_boot.trn_boot import _ntff_profile_via_ctypes
        h = _ntff_profile_via_ctypes("/opt/axon/libaxon_pjrt.so")
        if h is not None:
            mod.set_axon_ntff_profile_hook(h)
            import concourse.bass_utils as bu
            bu.upload_artifacts = lambda tmpdir: "local://" + str(tmpdir)
    except Exception:
        pass


def _strassen_operands(W1, X, bf16):
    """Host-side level-1 Strassen operand combos for H = W1^T X.

    W1: [D, FF] fp32; X: [D, C_PER] fp32. Returns (s_host, m_host):
      s_host [7, FTB, P, SDT, P] bf16 (stationary, w1 side)
      m_host [7, SDT, 2, P, CB] bf16 (moving, x side)
    """
    A = W1.T                                   # [FF, D]
    hf, hd, hc = FF // 2, D // 2, C_PER // 2
    A11, A12 = A[:hf, :hd], A[:hf, hd:]
    A21, A22 = A[hf:, :hd], A[hf:, hd:]
    B11, B12 = X[:hd, :hc], X[:hd, hc:]
    B21, B22 = X[hd:, :hc], X[hd:, hc:]
    S = [A11 + A22, A21 + A22, A11, A22, A11 + A12, A21 - A11, A12 - A22]
    M = [B11 + B22, B11, B12 - B22, B21 - B11, B22, B11 + B12, B21 + B22]
    # [k, ftb, dp, dt, fp] -> [ftb, dp, k, dt, fp] (contiguous per part.)
    s_host = np.stack([
        Sk.astype(bf16).reshape(FTB, P, SDT, P).transpose(0, 3, 2, 1)
        for Sk in S]).transpose(1, 2, 0, 3, 4)
    # [k, dt, cs, dp, c] -> [k, cs, dp, dt, c]
    m_host = np.stack([
        Mk.astype(bf16).reshape(SDT, P, 2, CB).transpose(0, 2, 1, 3)
        for Mk in M]).transpose(0, 2, 3, 1, 4)
    return np.ascontiguousarray(s_host), np.ascontiguousarray(m_host)


def kernel(x, router_w, w1, w2):
    import ml_dtypes
    from concourse import bass_utils
    _ensure_axon_hooks()

    xf = np.ascontiguousarray(x.reshape(-1, D), dtype=np.float32)
    rev, sw, sort_idx = _routing(xf, router_w)

    nc = _get_program()

    in_maps = []
    for e in range(E):
        rows = rev[e * C_PER:(e + 1) * C_PER]
        xct = np.ascontiguousarray(xf[rows].T)            # [D, C_PER] fp32
        s_host, m_host = _strassen_operands(w1[e], xct, ml_dtypes.bfloat16)
        w2b = np.ascontiguousarray(w2[e].astype(ml_dtypes.bfloat16))
        swt = np.ascontiguousarray(
            sw[e * C_PER:(e + 1) * C_PER].reshape(NCT, P).T)
        in_maps.append({"s7": s_host, "m7": m_host, "w2b": w2b,
                        "swt": swt})

    r = bass_utils.run_bass_kernel_spmd(nc, in_maps, core_ids=list(range(E)))
    _cache["last_result"] = r

    y_sorted = np.empty((NE, D), dtype=np.float32)
    for e in range(E):
        y_sorted[e * C_PER:(e + 1) * C_PER] = r.results[e]["y"].astype(
            np.float32)

    # invert the permutation and combine the top-2 contributions
    y_expanded = np.empty_like(y_sorted)
    y_expanded[sort_idx] = y_sorted
    out = y_expanded.reshape(N, TOP_K, D).sum(axis=1)
    return out.reshape(B, T, D)


# revision 50
# speedup vs baseline: 1.0091x; 1.0014x over previous
"""MoE feed-forward (top-2 routing, 8 experts) on 8 Trainium2 NeuronCores.

Expert-parallel sharding: host computes the (tiny) router + argsort
permutation exactly as the reference does, gathers each expert's token
chunk, and sends chunk e + expert e's weights to core e. Each core runs
a dense FFN: y = gelu(x @ w1) @ w2, scaled by the per-row gate weight.
Host then inverts the permutation and sums the top-2 contributions.

Device kernel (per core, SPMD), v3:
  - mm1 uses one level of Strassen (7 products instead of 8 on 2x2x2
    blocking of [4096f x 1024d] @ [1024d x 2048c]): 12.5% fewer PE
    cycles. The 7 stationary operand combinations (w1-side) and 7 moving
    combinations (x-side) are precomputed on the host and shipped as
    inputs; the P1..P7 -> H block recombination runs on the (otherwise
    idle) DVE, reading the 7 PSUM banks and writing fp32 SBUF scratch
    that ScalarE gelus into the bf16 h tile.
  - warmup matmuls on a memset tile during the initial DMA window so the
    PE HAM clock-gate reaches 8/8 before real work arrives.
  - mm2 dense in bf16 (w2 resident in SBUF): y[c,d] += h^T @ w2,
    accumulated over all 32 f-tiles in PSUM; gate scale applied during
    the PSUM->SBUF drain on ScalarE; y written out in bf16 (host
    combines in fp32).
  - phase order: mm1 units cs=0 (produces h for c-blocks 0 and 2) ->
    mm2 cb0+cb2 -> mm1 units cs=1 -> mm2 cb1+cb3.
"""

import os
import sys

# Register the CPU jax backend alongside whatever platform is configured,
# so the router can be computed on CPU (bit-exact with a CPU-evaluated
# reference). A no-op if jax is already initialized or cpu is present.
if "jax" not in sys.modules:
    _jp = os.environ.get("JAX_PLATFORMS", "")
    if _jp and "cpu" not in _jp.split(","):
        os.environ["JAX_PLATFORMS"] = _jp + ",cpu"

import numpy as np

# Static problem config
B, T, D, FF, E, TOP_K = 4, 2048, 1024, 4096, 8, 2
N = B * T                    # 8192 tokens
NE = N * TOP_K               # 16384 expanded rows
C_PER = NE // E              # 2048 rows per core / expert chunk
P = 128
FT = FF // P                 # 32 f-tiles
DT = D // P                  # 8 d-tiles
NCB = 4                      # c-blocks per core
CB = C_PER // NCB            # 512 tokens per c-block
NCT = C_PER // P             # 16 c-tiles of 128
FTB = FT // 2                # 16 f-tiles per Strassen block row
SDT = 4                      # d-tiles per Strassen half (512/128)
NWARM = 72                   # warmup matmuls (FD=128): sized so the PE-idle
                             # gap between warmup end (~14us) and first-unit
                             # data (~16us) stays under the ~3.4us HAM MID
                             # window - otherwise the clock re-throttles to
                             # 1.2GHz right as real work begins

_cache = {}


def _build_program(act_name="Gelu"):
    import concourse.mybir as mybir
    import concourse.tile as tile
    from concourse import bacc

    f32 = mybir.dt.float32
    bf16 = mybir.dt.bfloat16
    Act = mybir.ActivationFunctionType
    Alu = mybir.AluOpType

    nc = bacc.Bacc("TRN2", num_devices=E)
    # 7 stationary combos [ftb, dp, k, dt, fp] and 7 moving combos
    # [k, cs, dp, dt, c] — ordered so each DMA reads multi-KB
    # contiguous runs per partition (small descriptors tank HBM DMA)
    s_d = nc.dram_tensor("s7", [FTB, P, 7, SDT, P], bf16,
                         kind="ExternalInput")
    m_d = nc.dram_tensor("m7", [7, 2, P, SDT, CB], bf16,
                         kind="ExternalInput")
    w2_d = nc.dram_tensor("w2b", [FF, D], bf16, kind="ExternalInput")
    sw_d = nc.dram_tensor("swt", [P, NCT], f32, kind="ExternalInput")
    y_d = nc.dram_tensor("y", [C_PER, D], bf16, kind="ExternalOutput")

    with tile.TileContext(nc) as tc:
        with tc.tile_pool(name="const", bufs=1) as cpool, \
             tc.tile_pool(name="mp", bufs=1) as mp, \
             tc.tile_pool(name="sp", bufs=3) as sp, \
             tc.tile_pool(name="w2p", bufs=1) as w2p, \
             tc.tile_pool(name="hp", bufs=1) as hp, \
             tc.tile_pool(name="scr", bufs=1) as scr, \
             tc.tile_pool(name="yop", bufs=2) as yop, \
             tc.tile_pool(name="ps", bufs=1, space="PSUM") as ps:

            # ---- warmup: keep the PE busy while the first DMAs land ----
            wmt = cpool.tile([P, P], bf16, tag="warm_sb")
            nc.gpsimd.memset(wmt, 0.0)
            wps = ps.tile([P, CB], f32, tag="ps0", name="warm_ps")
            for _ in range(NWARM):
                nc.tensor.matmul(wps[:, :P], wmt, wmt, start=True, stop=True)

            swt = cpool.tile([P, NCT], f32, tag="swt")
            nc.scalar.dma_start(out=swt, in_=sw_d.ap())

            # moving combos for one cs: 28 resident [P, CB] tiles,
            # reloaded between phases (bufs=1: refill waits for the
            # last phase-A reader, streams during the mm2 phase)
            m_t = {}

            def load_m(cs):
                for k in range(7):
                    t = mp.tile([P, SDT, CB], bf16, tag=f"m{k}",
                                name=f"m{k}_{cs}")
                    nc.scalar.dma_start(out=t, in_=m_d.ap()[k, cs])
                    m_t[k] = t

            # w2 resident as 4 chunk tiles [P, 8, D], loaded in 0.5MB
            # slices on the sync queue interleaved with the stationary
            # stream: queue FIFO order actually delays them past the
            # startup burst (a queue with no other work issues its DMAs
            # immediately, whatever the program position)
            w2src = w2_d.ap().rearrange(
                "(q f8 p) d -> q p f8 d", q=4, f8=FT // 4, p=P)
            w2t = [w2p.tile([P, FT // 4, D], bf16, tag=f"w2_{i}",
                            name=f"w2_{i}") for i in range(4)]

            def load_w2_part(j):
                q, part = j // 4, j % 4
                nc.sync.dma_start(
                    out=w2t[q][:, 2 * part:2 * part + 2, :],
                    in_=w2src[q][:, 2 * part:2 * part + 2, :])

            def stt(out, in0, scalar, in1):
                nc.vector.scalar_tensor_tensor(
                    out=out, in0=in0, scalar=scalar, in1=in1,
                    op0=Alu.mult, op1=Alu.add)

            load_m(0)
            for cs in range(2):
                h_lo = hp.tile([P, FT, CB], bf16, tag="h_lo",
                               name=f"h_lo_{cs}")
                h_hi = hp.tile([P, FT, CB], bf16, tag="h_hi",
                               name=f"h_hi_{cs}")
                for ftb in range(FTB):
                    # all 7 stationary combos for this ftb in one DMA
                    s_t = sp.tile([P, 7, SDT, P], bf16, tag="s",
                                  name=f"s_{cs}_{ftb}")
                    nc.sync.dma_start(out=s_t, in_=s_d.ap()[ftb])
                    # second half of phase A: HBM is free of the m-load
                    # burst by then, and mm2 needs w2 only at ~110us
                    if cs == 0 and 6 <= ftb <= 13:
                        load_w2_part(2 * (ftb - 6))
                        load_w2_part(2 * (ftb - 6) + 1)
                    p_t = []
                    for k in range(7):
                        pk = ps.tile([P, CB], f32, tag=f"ps{k}",
                                     name=f"p{k}_{ftb}")
                        for dt in range(SDT):
                            nc.tensor.matmul(pk, s_t[:, k, dt, :],
                                             m_t[k][:, dt, :],
                                             start=(dt == 0),
                                             stop=(dt == SDT - 1))
                        p_t.append(pk)
                    def st(tag):
                        return scr.tile([P, CB], f32, tag=tag, name=tag)

                    dstA = h_lo[:, ftb, :]
                    dstC = h_lo[:, FTB + ftb, :]
                    dstB = h_hi[:, ftb, :]
                    dstD = h_hi[:, FTB + ftb, :]

                    def add_ps(out, sb, pk, sign=1.0):
                        # out = sb + sign*pk   (pk in PSUM; a 2-input
                        # DVE op may read at most ONE PSUM operand)
                        op1 = Alu.add if sign > 0 else Alu.subtract
                        nc.vector.scalar_tensor_tensor(
                            out=out, in0=sb, scalar=1.0, in1=pk,
                            op0=Alu.mult, op1=op1)

                    # H11 = P0+P3-P4+P6; H21 = P1+P3; H12 = P2+P4;
                    # H22 = P0-P1+P2+P5.
                    # DVE is strict FIFO: passes are ordered by when
                    # their newest PSUM input completes (P_k done at
                    # ~0.86*(k+1)us into the unit), so no pass
                    # head-of-line-blocks ready work behind it. hsA
                    # (needs P6, the last product) therefore goes last.
                    u0, u1, u2 = st("u0"), st("u1"), st("u2")
                    nc.vector.tensor_copy(u0, p_t[0])
                    nc.vector.tensor_copy(u1, p_t[1])
                    nc.vector.tensor_copy(u2, p_t[2])
                    v1, w1_, hsA = st("v1"), st("w1"), st("hsA")
                    hsC, hsB = st("hsC"), st("hsB")
                    v2, v3, hsD = st("v2"), st("v3"), st("hsD")
                    add_ps(v1, u0, p_t[3])
                    add_ps(hsC, u1, p_t[3])
                    nc.scalar.activation(dstC, hsC,
                                         getattr(Act, act_name))
                    add_ps(w1_, v1, p_t[4], -1.0)
                    add_ps(hsB, u2, p_t[4])
                    nc.scalar.activation(dstB, hsB,
                                         getattr(Act, act_name))
                    add_ps(v2, u0, p_t[1], -1.0)
                    add_ps(v3, v2, p_t[2])
                    add_ps(hsD, v3, p_t[5])
                    nc.scalar.activation(dstD, hsD,
                                         getattr(Act, act_name))
                    add_ps(hsA, w1_, p_t[6])
                    nc.scalar.activation(dstA, hsA,
                                         getattr(Act, act_name))

                if cs == 0:
                    # refill the moving combos for phase B; the WAR
                    # dependency on phase A's last reader makes these
                    # stream during the first mm2 phase
                    load_m(1)

                # mm2 for c-blocks cs (from h_lo) and 2+cs (from h_hi)
                for cb, h_t in ((cs, h_lo), (2 + cs, h_hi)):
                    for ct in range(CB // P):
                        g = cb * (CB // P) + ct
                        for db in range(2):
                            d0 = db * (D // 2)
                            yps = ps.tile([P, D // 2], f32,
                                          tag=f"ps{(ct * 2 + db) % 4}",
                                          name="yps")
                            for ft in range(FT):
                                nc.tensor.matmul(
                                    yps, h_t[:, ft, ct * P:(ct + 1) * P],
                                    w2t[ft // (FT // 4)][:, ft % (FT // 4),
                                                         d0:d0 + D // 2],
                                    start=(ft == 0), stop=(ft == FT - 1))
                            yo = yop.tile([P, D // 2], bf16, tag="yo",
                                          name="yo")
                            nc.scalar.activation(yo, yps, Act.Copy,
                                                 scale=swt[:, g:g + 1])
                            # per-half store: the last store overlaps the
                            # final matmul group instead of trailing it
                            # (scalar queue keeps sync free for the next
                            # phase's stationary-combo prefetch)
                            nc.scalar.dma_start(
                                out=y_d.ap()[g * P:(g + 1) * P,
                                             d0:d0 + D // 2], in_=yo)
    nc.compile()
    return nc


def _get_program():
    if "nc" not in _cache:
        _cache["nc"] = _build_program()
    return _cache["nc"]


def _routing(xf, router_w):
    """Replicate the reference gating bit-exactly where it matters.

    Returns (rev, sw, sort_idx). The top-k *selection* must match the
    reference exactly (it is discrete); we therefore compute the router
    logits with jax when available, mirroring reference.py. The softmax
    and sort bookkeeping is continuous or exactly replicable in numpy.
    """
    try:
        import jax
        import jax.numpy as jnp

        def _gate():
            logits = jnp.asarray(xf) @ jnp.asarray(router_w).T
            return jax.lax.top_k(logits, TOP_K)

        try:
            cpu = jax.devices("cpu")[0]
            with jax.default_device(cpu):
                tv, ti = _gate()
        except Exception:
            tv, ti = _gate()
        topv = np.asarray(tv, dtype=np.float32)
        topi = np.asarray(ti)
    except Exception:
        logits = xf @ router_w.T
        # top-2 with jax tie-breaking (lower index wins)
        i0 = np.argmax(logits, axis=-1)
        v0 = np.take_along_axis(logits, i0[:, None], axis=-1)[:, 0]
        masked = logits.copy()
        np.put_along_axis(masked, i0[:, None], -np.inf, axis=-1)
        i1 = np.argmax(masked, axis=-1)
        v1 = np.take_along_axis(masked, i1[:, None], axis=-1)[:, 0]
        topi = np.stack([i0, i1], axis=-1)
        topv = np.stack([v0, v1], axis=-1).astype(np.float32)

    # softmax over the two gate logits, float32
    m = topv.max(axis=-1, keepdims=True)
    e = np.exp(topv - m, dtype=np.float32)
    topw = (e / e.sum(axis=-1, keepdims=True)).astype(np.float32)

    idx_flat = topi.reshape(-1)
    w_flat = topw.reshape(-1)
    # stable argsort of integer keys is uniquely determined by the keys
    sort_idx = np.argsort(idx_flat, kind="stable")
    src = np.repeat(np.arange(N), TOP_K)
    rev = src[sort_idx]
    sw = w_flat[sort_idx]
    return rev, sw, sort_idx


def _ensure_axon_hooks():
    """Make `antenv.axon_hooks` importable so run_bass_kernel_spmd's
    trace path degrades gracefully (or works, if the axon boot shim is
    available) instead of crashing on ImportError."""
    try:
        import antenv.axon_hooks  # noqa: F401
        return
    except ImportError:
        pass
    import sys
    import types
    mod = types.ModuleType("antenv.axon_hooks")
    state = {"hook": None}
    mod.set_axon_ntff_profile_hook = lambda h: state.update(hook=h)
    mod.get_axon_ntff_profile_hook = lambda: state["hook"]
    try:
        import antenv
        sys.modules["antenv.axon_hooks"] = mod
        antenv.axon_hooks = mod
    except ImportError:
        return
    try:
        from trn_agent_boot.trn_boot import _ntff_profile_via_ctypes
        h = _ntff_profile_via_ctypes("/opt/axon/libaxon_pjrt.so")
        if h is not None:
            mod.set_axon_ntff_profile_hook(h)
            import concourse.bass_utils as bu
            bu.upload_artifacts = lambda tmpdir: "local://" + str(tmpdir)
    except Exception:
        pass


def _strassen_operands(W1, X, bf16):
    """Host-side level-1 Strassen operand combos for H = W1^T X.

    W1: [D, FF] fp32; X: [D, C_PER] fp32. Returns (s_host, m_host):
      s_host [7, FTB, P, SDT, P] bf16 (stationary, w1 side)
      m_host [7, SDT, 2, P, CB] bf16 (moving, x side)
    """
    A = W1.T                                   # [FF, D]
    hf, hd, hc = FF // 2, D // 2, C_PER // 2
    A11, A12 = A[:hf, :hd], A[:hf, hd:]
    A21, A22 = A[hf:, :hd], A[hf:, hd:]
    B11, B12 = X[:hd, :hc], X[:hd, hc:]
    B21, B22 = X[hd:, :hc], X[hd:, hc:]
    S = [A11 + A22, A21 + A22, A11, A22, A11 + A12, A21 - A11, A12 - A22]
    M = [B11 + B22, B11, B12 - B22, B21 - B11, B22, B11 + B12, B21 + B22]
    # [k, ftb, dp, dt, fp] -> [ftb, dp, k, dt, fp] (contiguous per part.)
    s_host = np.stack([
        Sk.astype(bf16).reshape(FTB, P, SDT, P).transpose(0, 3, 2, 1)
        for Sk in S]).transpose(1, 2, 0, 3, 4)
    # [k, dt, cs, dp, c] -> [k, cs, dp, dt, c]
    m_host = np.stack([
        Mk.astype(bf16).reshape(SDT, P, 2, CB).transpose(0, 2, 1, 3)
        for Mk in M]).transpose(0, 2, 3, 1, 4)
    return np.ascontiguousarray(s_host), np.ascontiguousarray(m_host)


def kernel(x, router_w, w1, w2):
    import ml_dtypes
    from concourse import bass_utils
    _ensure_axon_hooks()

    xf = np.ascontiguousarray(x.reshape(-1, D), dtype=np.float32)
    rev, sw, sort_idx = _routing(xf, router_w)

    nc = _get_program()

    in_maps = []
    for e in range(E):
        rows = rev[e * C_PER:(e + 1) * C_PER]
        xct = np.ascontiguousarray(xf[rows].T)            # [D, C_PER] fp32
        s_host, m_host = _strassen_operands(w1[e], xct, ml_dtypes.bfloat16)
        w2b = np.ascontiguousarray(w2[e].astype(ml_dtypes.bfloat16))
        swt = np.ascontiguousarray(
            sw[e * C_PER:(e + 1) * C_PER].reshape(NCT, P).T)
        in_maps.append({"s7": s_host, "m7": m_host, "w2b": w2b,
                        "swt": swt})

    r = bass_utils.run_bass_kernel_spmd(nc, in_maps, core_ids=list(range(E)))
    _cache["last_result"] = r

    y_sorted = np.empty((NE, D), dtype=np.float32)
    for e in range(E):
        y_sorted[e * C_PER:(e + 1) * C_PER] = r.results[e]["y"].astype(
            np.float32)

    # invert the permutation and combine the top-2 contributions
    y_expanded = np.empty_like(y_sorted)
    y_expanded[sort_idx] = y_sorted
    out = y_expanded.reshape(N, TOP_K, D).sum(axis=1)
    return out.reshape(B, T, D)
